# revision 3
# baseline (speedup 1.0000x reference)
"""CQAttention (BiDAF context-query attention) forward kernel for 8 Trainium2
NeuronCores.

Full inputs: context (64,128,1024) f32, question (64,128,128) f32, w (384,) f32.
Full output: (64, 512, 1024) f32.

Sharding: pure data parallel over batch — 8 batches per core, w replicated.

Math (per batch, X = context[b] (H,C), Y = question[b] (H,Q), w=(wq,wc,wcq)):
    S^T = (wcq*Y + wc 1^T)^T @ X              # (Q,C); wq term is softmax-invariant
    P   = exp(S^T)                            # unnormalized softmax numerators
    d   = rowsum(P); r = 1/d                  # softmax denominators (per q-row)
    A   = (diag(r) Y^T)^T @ P                 # = a^T                (H,C)
    tt  = P @ X^T                             # (Q,H) via PE transposes of P,X
    Bm  = (diag(r^2) tt)^T @ P                # = b^T = (s1 (s1^T c))^T  (H,C)
    out = [X; A; X*A; X*Bm]                   # (4H, C)

Matmul path runs in float32r (TF32-like). The three computed output blocks
(A, X*A, X*B) are stored to HBM as bfloat16 and upcast to f32 on the host —
this halves the dominant HBM write traffic while adding only an output-side
rounding (~0.4% per element) on top of the f32r compute error. Elementwise
work is spread over ACT / DVE / GpSimd; X*B reads B straight from PSUM so B
is never evacuated.
"""

import os
import sys

import numpy as np

if "/opt/trn_rl_repo" not in sys.path:
    sys.path.insert(0, "/opt/trn_rl_repo")

B, H, C, Q = 64, 128, 1024, 128
NCORES = 8
BPC = B // NCORES  # batches per core


def _ensure_ntff_hook():
    """This container's `antenv` stub lacks `axon_hooks`, which
    bass_utils needs for NTFF profiling under axon (trace=True). Install
    a functional shadow module + register the ctypes-based hook."""
    import types

    try:
        from antenv.axon_hooks import get_axon_ntff_profile_hook  # noqa: F401

        return  # real module present
    except ImportError:
        pass
    try:
        import antenv

        mod = types.ModuleType("antenv.axon_hooks")
        _state = {"hook": None}

        def set_axon_ntff_profile_hook(h):
            _state["hook"] = h

        def get_axon_ntff_profile_hook():
            return _state["hook"]

        mod.set_axon_ntff_profile_hook = set_axon_ntff_profile_hook
        mod.get_axon_ntff_profile_hook = get_axon_ntff_profile_hook
        sys.modules["antenv.axon_hooks"] = mod
        antenv.axon_hooks = mod

        from trn_agent_boot.trn_boot import _ntff_profile_via_ctypes

        set_axon_ntff_profile_hook(
            _ntff_profile_via_ctypes("/opt/axon/libaxon_pjrt.so")
        )
    except Exception:
        pass  # profiling degrades; compute still works


_ensure_ntff_hook()

LAST_RESULTS = None
_NC = None


def _build():
    from contextlib import ExitStack

    import concourse.bacc as bacc
    import concourse.mybir as mybir
    import concourse.tile as tile
    from concourse import masks

    f32 = mybir.dt.float32
    f32r = mybir.dt.float32r
    bf16 = mybir.dt.bfloat16
    EXP = mybir.ActivationFunctionType.Exp
    COPY = mybir.ActivationFunctionType.Copy
    MULT = mybir.AluOpType.mult
    ADD = mybir.AluOpType.add

    nc = bacc.Bacc(
        "TRN2", target_bir_lowering=False, debug=False, enable_asserts=False
    )
    ctx_t = nc.dram_tensor("context", (BPC, H, C), f32, kind="ExternalInput").ap()
    q_t = nc.dram_tensor("question", (BPC, H, Q), f32, kind="ExternalInput").ap()
    w_t = nc.dram_tensor("w", (3 * H,), f32, kind="ExternalInput").ap()
    # device writes blocks (A, X*A, X*B) as bf16; block 0 == context is
    # filled host-side during unshard (pure passthrough of an input).
    out_t = nc.dram_tensor("out", (BPC, 3, H, C), bf16, kind="ExternalOutput").ap()

    with tile.TileContext(nc) as tc, ExitStack() as ctx:
        const = ctx.enter_context(tc.tile_pool(name="const", bufs=1))
        sb = ctx.enter_context(tc.tile_pool(name="sb", bufs=3))
        sbx = ctx.enter_context(tc.tile_pool(name="sbx", bufs=3))
        sb3 = ctx.enter_context(tc.tile_pool(name="sb3", bufs=3))
        ps_tr = ctx.enter_context(tc.tile_pool(name="ptr", bufs=6, space="PSUM"))
        ps_tt = ctx.enter_context(tc.tile_pool(name="ptt", bufs=2, space="PSUM"))

        ident = const.tile([128, 128], f32, tag="ident")
        masks.make_identity(nc, ident[:])
        identr = const.tile([128, 128], f32r, tag="identr")
        nc.vector.tensor_copy(identr[:], ident[:])

        # w arrives as one contiguous (1,384) row; (128,1) columns via K=1
        # PE matmuls against identity (avoids slow scatter DMAs).
        w_row = const.tile([1, 3 * H], f32r, tag="w_row")
        nc.sync.dma_start(w_row[:], w_t.unsqueeze(0).bitcast(f32r))
        wc = const.tile([128, 1], f32, tag="wc")
        wcq = const.tile([128, 1], f32, tag="wcq")

        # All 8 questions in one DMA: Yall[h, b*Q+q] = question[b, h, q]
        Yall = const.tile([128, BPC * Q], f32, tag="Yall")
        nc.scalar.dma_start(
            Yall[:].rearrange("p (b q) -> p b q", b=BPC),
            q_t.transpose([1, 0, 2]),
        )
        # Rounded copy + all Y^T upfront: YTall[q, b*H+h] = Yr[b][h, q]
        Yrall = const.tile([128, BPC * Q], f32r, tag="Yrall")
        YTall = const.tile([128, BPC * H], f32r, tag="YTall")

        state = {}  # live tiles of the in-flight batch

        def front(b):
            # X kept in exact f32 (for X*A / X*B); Xr is the rounded f32r
            # copy for matmul/transpose paths.
            X = sbx.tile([H, C], f32, tag="X")
            if b == 0:
                nc.sync.dma_start(X[:, 0:512], ctx_t[b, :, 0:512])
                nc.sync.dma_start(X[:, 512:1024], ctx_t[b, :, 512:1024])
            else:
                nc.sync.dma_start(X[:], ctx_t[b])
            Xr = sbx.tile([H, C], f32r, tag="Xr")
            nc.vector.tensor_copy(Xr[:, 0:512], X[:, 0:512])
            nc.vector.tensor_copy(Xr[:, 512:1024], X[:, 512:1024])
            if b == 0:
                wps = ps_tr.tile([128, 512], f32, tag="tr")
                nc.tensor.matmul(
                    wps[:, 0:128],
                    w_row[0:1, H : 2 * H],
                    identr[0:1, 0:128],
                    start=True,
                    stop=True,
                )
                nc.tensor.matmul(
                    wps[:, 128:256],
                    w_row[0:1, 2 * H : 3 * H],
                    identr[0:1, 0:128],
                    start=True,
                    stop=True,
                )
                nc.vector.tensor_copy(wc[:], wps[:, 0:1])
                nc.vector.tensor_copy(wcq[:], wps[:, 128:129])
                # Yr (rounded) for the Y^T transposes
                nc.vector.tensor_copy(Yrall[:, 0:512], Yall[:, 0:512])
                nc.vector.tensor_copy(Yrall[:, 512:1024], Yall[:, 512:1024])
                for g in range(2):
                    ytp = ps_tr.tile([128, 512], f32, tag="tr")
                    for k in range(4):
                        bb = g * 4 + k
                        nc.tensor.transpose(
                            ytp[:, k * 128 : (k + 1) * 128].bitcast(f32r),
                            Yrall[:, bb * Q : (bb + 1) * Q],
                            identr[:],
                        )
                    nc.vector.tensor_copy(
                        YTall[:, g * 512 : (g + 1) * 512], ytp[:].bitcast(f32r)
                    )

            # Z = wcq * Y + wc  (so Z^T @ X = softmax logits S^T)
            Z = sb.tile([H, Q], f32r, tag="Z")
            nc.vector.tensor_scalar(
                Z[:], Yall[:, b * Q : (b + 1) * Q], wcq[:], wc[:], MULT, ADD
            )

            # scores + exp, in two 512-wide halves for pipelining
            P = sb.tile([Q, C], f32r, tag="P")
            dh = sb.tile([Q, 2], f32, tag="dh")
            for j in range(2):
                Sh = ps_tr.tile([Q, 512], f32, tag="tr")
                nc.tensor.matmul(
                    Sh[:], Z[:], Xr[:, j * 512 : (j + 1) * 512], start=True, stop=True
                )
                nc.scalar.activation(
                    P[:, j * 512 : (j + 1) * 512],
                    Sh[:],
                    EXP,
                    accum_out=dh[:, j : j + 1],
                )
            dsum = sb.tile([Q, 1], f32, tag="dsum")
            nc.vector.tensor_add(dsum[:], dh[:, 0:1], dh[:, 1:2])
            rr = sb.tile([Q, 1], f32, tag="rr")
            nc.vector.reciprocal(rr[:], dsum[:])
            r2 = sb.tile([Q, 1], f32, tag="r2")
            nc.vector.tensor_mul(r2[:], rr[:], rr[:])

            # XT holds [YTs | X^T chunks 0..7]; the leading YTs block means
            # every N=256 tt-matmul window reads initialized data.
            XT = sb.tile([128, 128 + C], f32r, tag="XT")
            nc.vector.tensor_scalar_mul(
                XT[:, 0:128], YTall[:, b * H : (b + 1) * H], rr[:]
            )

            state.update(X=X, Xr=Xr, P=P, rr=rr, r2=r2, XT=XT, b=b)

        def mid(b):
            X, P, XT, r2 = state["Xr"], state["P"], state["XT"], state["r2"]
            YTs = XT[:, 0:128]
            # X^T chunks first (independent of exp), then P^T chunks
            for g in range(2):
                xtp = ps_tr.tile([128, 512], f32, tag="tr")
                for k in range(4):
                    c0 = g * 4 + k
                    nc.tensor.transpose(
                        xtp[:, k * 128 : (k + 1) * 128].bitcast(f32r),
                        X[:, c0 * 128 : (c0 + 1) * 128],
                        identr[:],
                    )
                nc.scalar.copy(XT[:, 128 + g * 512 : 128 + (g + 1) * 512], xtp[:])

            PT = sb.tile([128, C], f32r, tag="PT")
            for g in range(2):
                ptp = ps_tr.tile([128, 512], f32, tag="tr")
                for k in range(4):
                    c0 = g * 4 + k
                    nc.tensor.transpose(
                        ptp[:, k * 128 : (k + 1) * 128].bitcast(f32r),
                        P[:, c0 * 128 : (c0 + 1) * 128],
                        identr[:],
                    )
                if g == 0:
                    nc.scalar.copy(PT[:, 0:512], ptp[:])
                else:
                    nc.vector.tensor_copy(PT[:, 512:1024], ptp[:].bitcast(f32r))

            # tt[:,128:256] = P @ X^T  (cols 0:128 accumulate junk, never read)
            tt = ps_tt.tile([Q, 256], f32, tag="tt")
            for c in range(8):
                nc.tensor.matmul(
                    tt[:],
                    PT[:, c * 128 : (c + 1) * 128],
                    XT[:, c * 128 : c * 128 + 256],
                    start=(c == 0),
                    stop=(c == 7),
                )
            tts = sb.tile([Q, H], f32r, tag="tts")
            nc.vector.tensor_scalar_mul(tts[:], tt[:, 128:256], r2[:])

            # A matmuls; evacuate straight into the bf16 store tile on ACT.
            S3 = sb3.tile([H, 3 * C], bf16, tag="S3")
            for j in range(2):
                Aps = ps_tr.tile([H, 512], f32, tag="tr")
                nc.tensor.matmul(
                    Aps[:], YTs, P[:, j * 512 : (j + 1) * 512], start=True, stop=True
                )
                nc.scalar.activation(S3[:, j * 512 : (j + 1) * 512], Aps[:], COPY)
            state.update(tts=tts, S3=S3)

        def back(b):
            X, P, tts, S3 = state["X"], state["P"], state["tts"], state["S3"]
            last = b == BPC - 1
            # B matmuls; X*B reads B straight from PSUM on DVE (B itself is
            # never stored), X*A runs on GpSimd from the evacuated bf16 A.
            if last:
                # drain: fire each output chunk the moment it is ready
                nc.scalar.dma_start(out_t[b, 0], S3[:, 0:C])
                for j in range(2):
                    nc.gpsimd.tensor_mul(
                        S3[:, C + j * 512 : C + (j + 1) * 512],
                        X[:, j * 512 : (j + 1) * 512],
                        S3[:, j * 512 : (j + 1) * 512],
                    )
                    nc.scalar.dma_start(
                        out_t[b, 1, :, j * 512 : (j + 1) * 512],
                        S3[:, C + j * 512 : C + (j + 1) * 512],
                    )
            for j in range(2):
                Bps = ps_tr.tile([H, 512], f32, tag="tr")
                nc.tensor.matmul(
                    Bps[:], tts[:], P[:, j * 512 : (j + 1) * 512], start=True, stop=True
                )
                nc.vector.tensor_mul(
                    S3[:, 2 * C + j * 512 : 2 * C + (j + 1) * 512],
                    X[:, j * 512 : (j + 1) * 512],
                    Bps[:],
                )
                if last:
                    nc.sync.dma_start(
                        out_t[b, 2, :, j * 512 : (j + 1) * 512],
                        S3[:, 2 * C + j * 512 : 2 * C + (j + 1) * 512],
                    )
            if not last:
                nc.gpsimd.tensor_mul(S3[:, C : 2 * C], X[:], S3[:, 0:C])
                # [A | XA] in one 512 KB DMA; XB in a second
                nc.sync.dma_start(
                    out_t[b, 0:2].transpose([1, 0, 2]),
                    S3[:, 0 : 2 * C].rearrange("p (k c) -> p k c", k=2),
                )
                nc.scalar.dma_start(out_t[b, 2], S3[:, 2 * C : 3 * C])

        prev = None
        for b in range(BPC):
            front(b)
            if prev is not None:
                back_state = prev
                cur = dict(state)
                state.clear()
                state.update(back_state)
                back(b - 1)
                state.clear()
                state.update(cur)
            mid(b)
            prev = dict(state)
        back(BPC - 1)

    nc.compile()
    return nc


def kernel(context, question, w):
    global _NC, LAST_RESULTS
    from concourse import bass_utils

    if _NC is None:
        _NC = _build()

    context = np.ascontiguousarray(np.asarray(context), dtype=np.float32)
    question = np.ascontiguousarray(np.asarray(question), dtype=np.float32)
    w = np.ascontiguousarray(np.asarray(w), dtype=np.float32)

    in_maps = [
        {
            "context": context[c * BPC : (c + 1) * BPC],
            "question": question[c * BPC : (c + 1) * BPC],
            "w": w,
        }
        for c in range(NCORES)
    ]
    trace = bool(int(os.environ.get("KTRACE", "0")))
    LAST_RESULTS = bass_utils.run_bass_kernel_spmd(
        _NC, in_maps, core_ids=list(range(NCORES)), trace=trace
    )
    out = np.empty((B, 4 * H, C), dtype=np.float32)
    out[:, 0:H, :] = context
    for c in range(NCORES):
        blk = np.asarray(LAST_RESULTS.results[c]["out"]).astype(np.float32)
        out[c * BPC : (c + 1) * BPC, H:, :] = blk.reshape(BPC, 3 * H, C)
    return out


# revision 4
# speedup vs baseline: 1.1498x; 1.1498x over previous
"""CQAttention (BiDAF context-query attention) forward kernel for 8 Trainium2
NeuronCores.

Full inputs: context (64,128,1024) f32, question (64,128,128) f32, w (384,) f32.
Full output: (64, 512, 1024) f32.

Sharding: pure data parallel over batch — 8 batches per core, w replicated.

Math (per batch, X = context[b] (H,C), Y = question[b] (H,Q), w=(wq,wc,wcq)):
    S^T = (wcq*Y + wc 1^T)^T @ X              # (Q,C); wq term is softmax-invariant
    P   = exp(S^T)                            # unnormalized softmax numerators
    r   = 1/rowsum(P)                         # softmax denominators (per q-row)
    A   = (diag(r) Y^T)^T @ P                 # = a^T                (H,C)
    tt  = P @ X^T                             # (Q,H) via PE transposes of P,X
    Bm  = (diag(r^2) tt)^T @ P                # = b^T = (s1 (s1^T c))^T  (H,C)
    out = [X; A; X*A; X*Bm]                   # (4H, C)

The whole pipeline runs in bf16 (inputs are cast host-side; matmuls accumulate
in f32 PSUM; exp reads f32 scores and its row-sums accumulate in f32), and the
three computed output blocks are stored as bf16 and upcast host-side. Block 0
(== context) is filled host-side from the exact f32 input. Max-normalized
relative error lands ~2.4e-3 vs the 2e-2 gate. bf16 halves both HBM streams,
enables FWL weight loads, single-bank transpose tiles, and 2x-packed PSUM
evacuations.
"""

import os
import sys

import numpy as np

if "/opt/trn_rl_repo" not in sys.path:
    sys.path.insert(0, "/opt/trn_rl_repo")

B, H, C, Q = 64, 128, 1024, 128
NCORES = 8
BPC = B // NCORES  # batches per core


def _ensure_ntff_hook():
    """This container's `antenv` stub lacks `axon_hooks`, which
    bass_utils needs for NTFF profiling under axon (trace=True). Install
    a functional shadow module + register the ctypes-based hook."""
    import types

    try:
        from antenv.axon_hooks import get_axon_ntff_profile_hook  # noqa: F401

        return  # real module present
    except ImportError:
        pass
    try:
        import antenv

        mod = types.ModuleType("antenv.axon_hooks")
        _state = {"hook": None}

        def set_axon_ntff_profile_hook(h):
            _state["hook"] = h

        def get_axon_ntff_profile_hook():
            return _state["hook"]

        mod.set_axon_ntff_profile_hook = set_axon_ntff_profile_hook
        mod.get_axon_ntff_profile_hook = get_axon_ntff_profile_hook
        sys.modules["antenv.axon_hooks"] = mod
        antenv.axon_hooks = mod

        from trn_agent_boot.trn_boot import _ntff_profile_via_ctypes

        set_axon_ntff_profile_hook(
            _ntff_profile_via_ctypes("/opt/axon/libaxon_pjrt.so")
        )
    except Exception:
        pass  # profiling degrades; compute still works


_ensure_ntff_hook()

LAST_RESULTS = None
_NC = None


def _build():
    from contextlib import ExitStack

    import concourse.bacc as bacc
    import concourse.mybir as mybir
    import concourse.tile as tile
    from concourse import masks

    f32 = mybir.dt.float32
    f32r = mybir.dt.float32r
    bf16 = mybir.dt.bfloat16
    EXP = mybir.ActivationFunctionType.Exp
    COPY = mybir.ActivationFunctionType.Copy
    MULT = mybir.AluOpType.mult
    ADD = mybir.AluOpType.add

    nc = bacc.Bacc(
        "TRN2", target_bir_lowering=False, debug=False, enable_asserts=False
    )
    ctx_t = nc.dram_tensor("context", (BPC, H, C), bf16, kind="ExternalInput").ap()
    q_t = nc.dram_tensor("question", (BPC, H, Q), bf16, kind="ExternalInput").ap()
    w_t = nc.dram_tensor("w", (3 * H,), f32, kind="ExternalInput").ap()
    # device writes blocks (A, X*A, X*B) as bf16; block 0 == context is
    # filled host-side during unshard (pure passthrough of an input).
    out_t = nc.dram_tensor("out", (BPC, 3, H, C), bf16, kind="ExternalOutput").ap()

    with tile.TileContext(nc) as tc, ExitStack() as ctx:
        const = ctx.enter_context(tc.tile_pool(name="const", bufs=1))
        sb = ctx.enter_context(tc.tile_pool(name="sb", bufs=3))
        sbx = ctx.enter_context(tc.tile_pool(name="sbx", bufs=3))
        sb3 = ctx.enter_context(tc.tile_pool(name="sb3", bufs=3))
        # PSUM (8 banks): ps_s 1x[128,1024]f32 (2 banks, S scores),
        # ps_ab 3x[128,512]f32 (3 banks: A, B, w), ps_tp 2x[128,1024]bf16
        # (2 banks: X^T, P^T, yt at startup), ps_tt 1x[128,128]f32 (1 bank).
        ps_s = ctx.enter_context(tc.tile_pool(name="ps_s", bufs=1, space="PSUM"))
        ps_ab = ctx.enter_context(tc.tile_pool(name="ps_ab", bufs=3, space="PSUM"))
        ps_tp = ctx.enter_context(tc.tile_pool(name="ps_tp", bufs=2, space="PSUM"))
        ps_tt = ctx.enter_context(tc.tile_pool(name="ps_tt", bufs=1, space="PSUM"))

        ident = const.tile([128, 128], f32, tag="ident")
        masks.make_identity(nc, ident[:])
        identb = const.tile([128, 128], bf16, tag="identb")
        nc.vector.tensor_copy(identb[:], ident[:])
        identr = const.tile([128, 128], f32r, tag="identr")
        nc.gpsimd.tensor_copy(identr[:], ident[:])

        # w arrives as one contiguous (1,384) row; (128,1) columns via K=1
        # PE matmuls against identity (avoids slow scatter DMAs).
        w_row = const.tile([1, 3 * H], f32r, tag="w_row")
        nc.sync.dma_start(w_row[:], w_t.unsqueeze(0).bitcast(f32r))
        wc = const.tile([128, 1], f32, tag="wc")
        wcq = const.tile([128, 1], f32, tag="wcq")

        # All 8 questions in one DMA: Yall[h, b*Q+q] = question[b, h, q]
        Yall = const.tile([128, BPC * Q], bf16, tag="Yall")
        nc.sync.dma_start(
            Yall[:].rearrange("p (b q) -> p b q", b=BPC),
            q_t.transpose([1, 0, 2]),
        )
        # All Y^T upfront: YTall[q, b*H+h] = Y[b][h, q]
        YTall = const.tile([128, BPC * H], bf16, tag="YTall")

        state = {}  # live tiles of the in-flight batch

        def front(b):
            X = sbx.tile([H, C], bf16, tag="X")
            if b == 0:
                nc.sync.dma_start(X[:, 0:512], ctx_t[b, :, 0:512])
                nc.sync.dma_start(X[:, 512:1024], ctx_t[b, :, 512:1024])
            else:
                nc.sync.dma_start(X[:], ctx_t[b])
            if b == 0:
                wps = ps_ab.tile([128, 512], f32, tag="ab")
                nc.tensor.matmul(
                    wps[:, 0:128],
                    w_row[0:1, H : 2 * H],
                    identr[0:1, 0:128],
                    start=True,
                    stop=True,
                )
                nc.tensor.matmul(
                    wps[:, 128:256],
                    w_row[0:1, 2 * H : 3 * H],
                    identr[0:1, 0:128],
                    start=True,
                    stop=True,
                )
                nc.vector.tensor_copy(wc[:], wps[:, 0:1])
                nc.vector.tensor_copy(wcq[:], wps[:, 128:129])

            # Z = wcq * Y + wc  (so Z^T @ X = softmax logits S^T)
            Z = sb.tile([H, Q], bf16, tag="Z")
            nc.vector.tensor_scalar(
                Z[:], Yall[:, b * Q : (b + 1) * Q], wcq[:], wc[:], MULT, ADD
            )

            # scores (2 matmuls into one contiguous 2-bank tile) + 1 exp
            Sps = ps_s.tile([Q, C], f32, tag="s")
            for j in range(2):
                nc.tensor.matmul(
                    Sps[:, j * 512 : (j + 1) * 512],
                    Z[:],
                    X[:, j * 512 : (j + 1) * 512],
                    start=True,
                    stop=True,
                )
            P = sb.tile([Q, C], bf16, tag="P")
            dsum = sb.tile([Q, 1], f32, tag="dsum")
            nc.scalar.activation(P[:], Sps[:], EXP, accum_out=dsum[:])
            rr = sb.tile([Q, 1], f32, tag="rr")
            nc.vector.reciprocal(rr[:], dsum[:])
            r2 = sb.tile([Q, 1], f32, tag="r2")
            nc.vector.tensor_mul(r2[:], rr[:], rr[:])

            if b == 0:
                # All 8 Y^T transposes back-to-back into one bf16 bank
                ytp = ps_tp.tile([128, BPC * Q], bf16, tag="tp")
                for bb in range(BPC):
                    nc.tensor.transpose(
                        ytp[:, bb * Q : (bb + 1) * Q],
                        Yall[:, bb * Q : (bb + 1) * Q],
                        identb[:],
                    )
                nc.vector.tensor_copy(YTall[:], ytp[:])

            YTs = sb.tile([Q, H], bf16, tag="YTs")
            nc.vector.tensor_scalar_mul(
                YTs[:], YTall[:, b * H : (b + 1) * H], rr[:]
            )
            state.update(X=X, P=P, rr=rr, r2=r2, YTs=YTs, b=b)

        def mid(b):
            X, P, YTs, r2 = state["X"], state["P"], state["YTs"], state["r2"]
            # X^T / P^T: 8 transposes each into one single-bank bf16 tile,
            # evacuated with one 2x-packed copy (ACT for X^T, DVE for P^T).
            xtp = ps_tp.tile([128, C], bf16, tag="tp")
            for k in range(8):
                nc.tensor.transpose(
                    xtp[:, k * 128 : (k + 1) * 128],
                    X[:, k * 128 : (k + 1) * 128],
                    identb[:],
                )
            XT = sb.tile([128, C], bf16, tag="XT")
            nc.scalar.copy(XT[:], xtp[:])

            ptp = ps_tp.tile([128, C], bf16, tag="tp")
            for k in range(8):
                nc.tensor.transpose(
                    ptp[:, k * 128 : (k + 1) * 128],
                    P[:, k * 128 : (k + 1) * 128],
                    identb[:],
                )
            PT = sb.tile([128, C], bf16, tag="PT")
            nc.vector.tensor_copy(PT[:], ptp[:])

            # tt = P @ X^T, accumulated over the 8 c-chunks
            tt = ps_tt.tile([Q, H], f32, tag="tt")
            for k in range(8):
                nc.tensor.matmul(
                    tt[:],
                    PT[:, k * 128 : (k + 1) * 128],
                    XT[:, k * 128 : (k + 1) * 128],
                    start=(k == 0),
                    stop=(k == 7),
                )
            tts = sb.tile([Q, H], bf16, tag="tts")
            nc.vector.tensor_scalar_mul(tts[:], tt[:], r2[:])

            # A matmuls; evacuate straight into the bf16 store tile on ACT.
            S3 = sb3.tile([H, 3 * C], bf16, tag="S3")
            for j in range(2):
                Aps = ps_ab.tile([H, 512], f32, tag="ab")
                nc.tensor.matmul(
                    Aps[:], YTs[:], P[:, j * 512 : (j + 1) * 512], start=True, stop=True
                )
                nc.scalar.activation(S3[:, j * 512 : (j + 1) * 512], Aps[:], COPY)
            state.update(tts=tts, S3=S3)

        def back(b):
            X, P, tts, S3 = state["X"], state["P"], state["tts"], state["S3"]
            last = b == BPC - 1
            # B matmuls; X*B reads B straight from PSUM on DVE (B itself is
            # never stored), X*A runs on GpSimd from the evacuated bf16 A.
            if last:
                # drain: fire each output chunk the moment it is ready
                nc.sync.dma_start(out_t[b, 0], S3[:, 0:C])
                for j in range(2):
                    nc.gpsimd.tensor_mul(
                        S3[:, C + j * 512 : C + (j + 1) * 512],
                        X[:, j * 512 : (j + 1) * 512],
                        S3[:, j * 512 : (j + 1) * 512],
                    )
                    nc.sync.dma_start(
                        out_t[b, 1, :, j * 512 : (j + 1) * 512],
                        S3[:, C + j * 512 : C + (j + 1) * 512],
                    )
            for j in range(2):
                Bps = ps_ab.tile([H, 512], f32, tag="ab")
                nc.tensor.matmul(
                    Bps[:], tts[:], P[:, j * 512 : (j + 1) * 512], start=True, stop=True
                )
                nc.vector.tensor_mul(
                    S3[:, 2 * C + j * 512 : 2 * C + (j + 1) * 512],
                    X[:, j * 512 : (j + 1) * 512],
                    Bps[:],
                )
                if last:
                    nc.sync.dma_start(
                        out_t[b, 2, :, j * 512 : (j + 1) * 512],
                        S3[:, 2 * C + j * 512 : 2 * C + (j + 1) * 512],
                    )
            if not last:
                nc.gpsimd.tensor_mul(S3[:, C : 2 * C], X[:], S3[:, 0:C])
                # [A | XA] in one 512 KB DMA; XB in a second
                nc.sync.dma_start(
                    out_t[b, 0:2].transpose([1, 0, 2]),
                    S3[:, 0 : 2 * C].rearrange("p (k c) -> p k c", k=2),
                )
                nc.sync.dma_start(out_t[b, 2], S3[:, 2 * C : 3 * C])

        prev = None
        for b in range(BPC):
            front(b)
            if prev is not None:
                back_state = prev
                cur = dict(state)
                state.clear()
                state.update(back_state)
                back(b - 1)
                state.clear()
                state.update(cur)
            mid(b)
            prev = dict(state)
        back(BPC - 1)

    nc.compile()
    return nc


def kernel(context, question, w):
    global _NC, LAST_RESULTS
    import ml_dtypes

    from concourse import bass_utils

    if _NC is None:
        _NC = _build()

    context = np.ascontiguousarray(np.asarray(context), dtype=np.float32)
    question = np.ascontiguousarray(np.asarray(question), dtype=np.float32)
    w = np.ascontiguousarray(np.asarray(w), dtype=np.float32)
    ctx_bf = context.astype(ml_dtypes.bfloat16)
    q_bf = question.astype(ml_dtypes.bfloat16)

    in_maps = [
        {
            "context": ctx_bf[c * BPC : (c + 1) * BPC],
            "question": q_bf[c * BPC : (c + 1) * BPC],
            "w": w,
        }
        for c in range(NCORES)
    ]
    trace = bool(int(os.environ.get("KTRACE", "0")))
    LAST_RESULTS = bass_utils.run_bass_kernel_spmd(
        _NC, in_maps, core_ids=list(range(NCORES)), trace=trace
    )
    out = np.empty((B, 4 * H, C), dtype=np.float32)
    out[:, 0:H, :] = context
    for c in range(NCORES):
        blk = np.asarray(LAST_RESULTS.results[c]["out"]).astype(np.float32)
        out[c * BPC : (c + 1) * BPC, H:, :] = blk.reshape(BPC, 3 * H, C)
    return out
